# revision 1
# baseline (speedup 1.0000x reference)
"""Trainium2 Bass kernel for nn_CompositeEmbeddingA (octree composite embedding).

Three SPMD launches on 8 NeuronCores (all model compute on device; the host
only slices / concatenates arrays between launches):

  P1 (core = item*2+half): embedding gather-sum for every segment via
     dma_gather (bf16, channel-major) from on-device-built combined tables,
     first-level convs W4..W8a as bf16 matmuls, base/identity row exports via
     PE transposes.
  P2 (core = item*2+shalf): substituted sequences s7=sub(emb7,c8),
     s6=sub(emb6,c7a) assembled by one indexed gather from concatenated row
     sources (indices from on-device mask/cumsum), then convs W8b/W7b.
  P3 (core = item*2+ohalf): final d6/d7/d8 rows assembled by token-major
     indexed gathers (base rows vs substituted conv rows), upcast to f32.

Returns the full [4, 25672, 256] f32 output.
"""

import numpy as np
import ml_dtypes

import concourse.bass as bass
import concourse.tile as tile
from concourse import bacc, mybir
from concourse.masks import make_identity

BF16 = mybir.dt.bfloat16
F32 = mybir.dt.float32
I16 = mybir.dt.int16
I32 = mybir.dt.int32
AOP = mybir.AluOpType
ACT_IDENT = mybir.ActivationFunctionType.Identity
NPBF16 = ml_dtypes.bfloat16

P = 128
E = 256
B = 4
COUNTS = [8, 64, 256, 1024, 4096, 16384, 32768, 65536]
OFF = [0, 8, 72, 328, 1352, 5448, 21832, 54600, 120136]
S = OFF[-1]
NOUT = 25672
KSZ = {4: 4, 5: 8, 6: 8, 7: 8, 8: 8}
NQ = 1                # SWDGE queues used by dma_gather (set via num_swdge_queues)
TPAD = 384            # padded rows per pos-table block
T0_BLOCK = 3 * TPAD   # rows per depth in combined val+dep+pos0 table (1152)


def seg_val(a, d):
    return a[:, OFF[d - 1]:OFF[d]]


# ===========================================================================
# device-side helpers
# ===========================================================================

def build_tables(nc, sb, psb, dram, pos_pad, val_tab, dep_tab):
    """Build bf16 gather tables in DRAM:
       t0_all [8*1152, 256]: row (d-1)*1152+(v-1)*384+p = val[v]+dep[d]+pos0[p]
       t1, t2 [384, 256]: pos_tab[1], pos_tab[2] (rows 257..383 padding)."""
    t0_all = dram.tile([8 * T0_BLOCK, E], BF16)
    t1 = dram.tile([TPAD, E], BF16)
    t2 = dram.tile([TPAD, E], BF16)

    pos_sb = []
    for a in range(3):
        t = sb.tile([P, 3, E], F32, tag=f"pos_stage{a}")
        nc.sync.dma_start(t[:], pos_pad[a].rearrange("(j p) e -> p j e", p=P))
        pos_sb.append(t)

    for a, tdst in ((1, t1), (2, t2)):
        tb = sb.tile([P, 3, E], BF16, tag=f"pos_bf{a}")
        nc.vector.tensor_copy(tb[:], pos_sb[a][:])
        nc.sync.dma_start(tdst[:].rearrange("(j p) e -> p j e", p=P), tb[:])

    vt = sb.tile([1, 4 * E], F32, tag="vt")
    dt_ = sb.tile([1, 9 * E], F32, tag="dt")
    nc.sync.dma_start(vt[:], val_tab[:].rearrange("v e -> (v e)"))
    nc.sync.dma_start(dt_[:], dep_tab[:].rearrange("v e -> (v e)"))
    vd = sb.tile([1, 24 * E], F32, tag="vd")
    for d in range(1, 9):
        for v in range(1, 4):
            r = 3 * (d - 1) + (v - 1)
            nc.vector.tensor_tensor(
                vd[:, r * E:(r + 1) * E], vt[:, v * E:(v + 1) * E],
                dt_[:, d * E:(d + 1) * E], AOP.add)

    ones_f = sb.tile([1, P], F32, tag="ones_f")
    nc.vector.memset(ones_f[:], 1.0)
    for d in range(1, 9):
        stage = sb.tile([P, 9, E], BF16, tag="t0_stage")
        for v in range(1, 4):
            r = 3 * (d - 1) + (v - 1)
            bc = psb.tile([P, E], F32, tag="bc_ps")
            nc.tensor.matmul(bc[:], ones_f[:], vd[:, r * E:(r + 1) * E],
                             start=True, stop=True)
            for j in range(3):
                nc.vector.tensor_tensor(
                    stage[:, 3 * (v - 1) + j, :], pos_sb[0][:, j, :], bc[:],
                    AOP.add)
        nc.sync.dma_start(
            t0_all[(d - 1) * T0_BLOCK:d * T0_BLOCK, :].rearrange(
                "(j p) e -> p j e", p=P),
            stage[:])
    return t0_all, t1, t2


def replicate_idx(nc, sb, src16, width, tag):
    """[16, width] i16 tile -> [128, width] replicated (dma_gather idx)."""
    full = sb.tile([P, width], I16, tag=tag)
    for k in range(8):
        nc.sync.dma_start(full[16 * k:16 * (k + 1), :], src16[:, :])
    return full


def prep_gather_idx_p1(nc, sb, val_w, pos_w, m, d_list, tag):
    """Replicated idx tiles (idx0->t0_all, idx1->t1, idx2->t2) for m tokens.
    val_w: [16, m/16] i32 DRAM (host-wrapped, token t at [t%16, t//16]);
    pos_w: [3, 16, m/16] i32 DRAM.
    d_list: (depth, start, count) ranges, count % 16 == 0."""
    w = m // 16
    # i16-native loads (SWDGE dma casts i32 -> i16)
    v_t = sb.tile([16, w], I16, tag="ix_v")
    nc.gpsimd.dma_start(v_t[:], val_w)
    pts = []
    for a in range(3):
        t = sb.tile([16, w], I16, tag=f"ix_p{a}")
        nc.gpsimd.dma_start(t[:], pos_w[a])
        pts.append(t)

    i0 = sb.tile([16, w], I16, tag="ix_i0")
    nc.vector.tensor_scalar(i0[:], v_t[:], 1.0, float(TPAD),
                            AOP.subtract, AOP.mult)
    nc.vector.tensor_tensor(i0[:], i0[:], pts[0][:], AOP.add)
    for (d, start, count) in d_list:
        sl = i0[:, start // 16:(start + count) // 16]
        nc.vector.tensor_scalar(sl, sl, float((d - 1) * T0_BLOCK), None, AOP.add)

    return (replicate_idx(nc, sb, i0, w, "idxr0"),
            replicate_idx(nc, sb, pts[1], w, "idxr1"),
            replicate_idx(nc, sb, pts[2], w, "idxr2"))


def gather_emb_tile(nc, pool, tabs, idxs, tile_start, T, tag):
    """3x dma_gather (transpose, bf16) + 2 adds -> [128, 2, T] ch-major emb."""
    t0_all, t1, t2 = tabs
    i0, i1, i2 = idxs
    c0, c1 = tile_start // 16, (tile_start + T) // 16
    g = []
    for nm, (tab, idx) in enumerate(((t0_all, i0), (t1, i1), (t2, i2))):
        o = pool.tile([P, 2, T], BF16, tag=f"gg{nm}")
        nc.gpsimd.dma_gather(
            out_ap=o[:], in_ap=tab[:], idxs_ap=idx[:, c0:c1],
            num_idxs=T, num_idxs_reg=T, elem_size=E, transpose=True,
            single_packet=False, queue_num=nm % NQ)
        g.append(o)
    nc.vector.tensor_tensor(g[0][:], g[0][:], g[1][:], AOP.add)
    nc.vector.tensor_tensor(g[0][:], g[0][:], g[2][:], AOP.add)
    return g[0]


def load_conv_weights(nc, sb, w_dram, k, tag):
    """W [k,256,256] f32 -> bf16 SBUF [128, k, 2, 2, 128]
    (p = in-ch within half; dims: phase, in-half j, out-half m, out col)."""
    wf = sb.tile([P, k, 2, 2, P], F32, tag="w_stage")
    nc.sync.dma_start(
        wf[:], w_dram[:].rearrange("k (j p) (m c) -> p k j m c", p=P, c=P))
    wb = sb.tile([P, k, 2, 2, P], BF16, tag=tag + "_b")
    nc.vector.tensor_copy(wb[:], wf[:])
    return wb


def load_bias(nc, sb, b_dram, tag):
    bt = sb.tile([P, 2], F32, tag=tag)
    nc.sync.dma_start(bt[:], b_dram[:].rearrange("(m p) -> p m", p=P))
    return bt


def conv_tile(nc, pool, ps, emb, wb, bias, k, T, tag):
    """emb [128, 2, T] bf16 ch-major -> conv out bf16 ch-major [128, 2, T/k]."""
    G = T // k
    out = pool.tile([P, 2, G], BF16, tag="co")
    emb_r = emb[:, :, :].rearrange("p j (g k) -> p j k g", k=k)
    for mo in range(2):
        pt = ps.tile([P, G], F32, tag="cps")
        n_mm = 2 * k
        i = 0
        for j in range(2):
            for phase in range(k):
                nc.tensor.matmul(
                    pt[:], wb[:, phase, j, mo, :], emb_r[:, j, phase, :],
                    start=(i == 0), stop=(i == n_mm - 1))
                i += 1
        nc.scalar.activation(
            out[:, mo, :], pt[:], ACT_IDENT, bias=bias[:, mo:mo + 1], scale=1.0)
    return out


def transpose_export(nc, pool, ps, src, n_tok, dram_out, row_off, tag,
                     ident, out_f32=False):
    """src [128, 2, n_tok] bf16 ch-major -> token-major rows into
    dram_out[row_off:row_off+n_tok].  n_tok % 128 == 0."""
    nblk = n_tok // P
    for c0 in range(0, nblk, 4):
        nb = min(4, nblk - c0)
        pt = ps.tile([P, 4, E], BF16, tag="tps")
        for bi in range(nb):
            for j in range(2):
                nc.tensor.transpose(
                    pt[:, bi, j * P:(j + 1) * P],
                    src[:, j, (c0 + bi) * P:(c0 + bi + 1) * P],
                    ident[:])
        st = pool.tile([P, 4, E], F32 if out_f32 else BF16,
                        tag="st_f" if out_f32 else "st_b")
        nc.scalar.copy(st[:, :nb, :], pt[:, :nb, :])
        nc.sync.dma_start(
            dram_out[row_off + c0 * P: row_off + (c0 + nb) * P, :].rearrange(
                "(c p) e -> p c e", p=P),
            st[:, :nb, :])


def make_sub_helpers(nc, sb):
    """ut16 [16,16] bf16 upper-tri-incl-diag; ones16 [16,1] bf16;
    ones1_f [1,16] f32."""
    ones = sb.tile([16, 16], BF16, tag="ut_ones")
    nc.vector.memset(ones[:], 1.0)
    ut16 = sb.tile([16, 16], BF16, tag="ut16")
    nc.gpsimd.affine_select(
        ut16[:], ones[:], pattern=[[1, 16]], compare_op=AOP.is_ge,
        fill=0.0, base=0, channel_multiplier=-1)
    ones16 = sb.tile([16, 1], BF16, tag="ones16")
    nc.vector.memset(ones16[:], 1.0)
    ones1_f = sb.tile([1, 16], F32, tag="ones1f")
    nc.vector.memset(ones1_f[:], 1.0)
    return ut16, ones16, ones1_f


def sub_gather_idx(nc, sb, ps1, val_w, n, m, base_rows, ut16, ones16,
                   ones1_f, hsel_f, tag):
    """Indices for substitution gather (all in wrapped-16 layout).

    val_w: [16, n/16] i32 DRAM (host-wrapped, my half = first m/16 cols).
    For local t in [0, m):
        idx[t] = mask[t] ? base_rows + (lrank[t] + hsel*cnt_other) - 1 : t
    Returns replicated [128, m/16] i16 idx tile."""
    w = n // 16
    mw = m // 16
    vt = sb.tile([16, w], I16, tag="sx_v")
    nc.gpsimd.dma_start(vt[:], val_w)
    mask = sb.tile([16, w], F32, tag="sx_m")
    nc.vector.tensor_scalar(mask[:], vt[:], 2.0, None, AOP.is_equal)
    maskb = sb.tile([16, w], BF16, tag="sx_mb")
    nc.vector.tensor_copy(maskb[:], mask[:])
    # within-column inclusive prefix (tokens 16c..16c+15 live in column c)
    pref = sb.tile([16, w], F32, tag="sx_pf")
    cs_sb = sb.tile([1, w], F32, tag="sx_cb")
    for q0 in range(0, w, 512):
        qn = min(512, w - q0)
        pref_ps = ps1.tile([16, 512], F32, tag="sx_pp")
        nc.tensor.matmul(pref_ps[:, :qn], ut16[:], maskb[:, q0:q0 + qn],
                         start=True, stop=True)
        nc.vector.tensor_copy(pref[:, q0:q0 + qn], pref_ps[:, :qn])
        cs_ps = ps1.tile([1, 512], F32, tag="sx_cp")
        nc.tensor.matmul(cs_ps[:, :qn], ones16[:], maskb[:, q0:q0 + qn],
                         start=True, stop=True)
        nc.vector.tensor_copy(cs_sb[:, q0:q0 + qn], cs_ps[:, :qn])
    csum = sb.tile([1, w], F32, tag="sx_cs")
    zr = sb.tile([1, w], F32, tag="sx_zr")
    nc.vector.memset(zr[:], 0.0)
    nc.vector.tensor_tensor_scan(
        csum[:], cs_sb[:], zr[:], 0.0, AOP.add, AOP.add)
    # exclusive col offsets for my half + hsel*cnt_other
    cnt = sb.tile([1, 1], F32, tag="sx_cn")
    nc.vector.tensor_tensor(
        cnt[:], csum[:, w - 1:w], csum[:, mw - 1:mw], AOP.subtract)
    nc.vector.tensor_tensor(cnt[:], cnt[:], hsel_f[:], AOP.mult)
    offm = sb.tile([1, mw], F32, tag="sx_om")
    nc.vector.memset(offm[:, 0:1], 0.0)
    nc.vector.tensor_copy(offm[:, 1:mw], csum[:, 0:mw - 1])
    nc.vector.tensor_tensor(
        offm[:], offm[:], cnt[:].to_broadcast([1, mw]), AOP.add)
    # broadcast [1, mw] -> [16, mw] via f32 ones matmul (exact), chunked
    a = sb.tile([16, mw], F32, tag="sx_a")
    for q0 in range(0, mw, 512):
        qn = min(512, mw - q0)
        off_bc = ps1.tile([16, 512], F32, tag="sx_ob")
        nc.tensor.matmul(off_bc[:, :qn], ones1_f[:], offm[:, q0:q0 + qn],
                         start=True, stop=True)
        nc.vector.tensor_tensor(a[:, q0:q0 + qn], pref[:, q0:q0 + qn],
                                off_bc[:, :qn], AOP.add)
    nc.vector.tensor_scalar(a[:], a[:], float(base_rows - 1), None, AOP.add)
    # t values: t = 16c + p
    ti = sb.tile([16, mw], I32, tag="sx_ti")
    nc.gpsimd.iota(ti[:], pattern=[[16, mw]], base=0, channel_multiplier=1)
    tf = sb.tile([16, mw], F32, tag="sx_tf")
    nc.vector.tensor_copy(tf[:], ti[:])
    # idx = t + mask*(a - t)
    nc.vector.tensor_tensor(a[:], a[:], tf[:], AOP.subtract)
    nc.vector.tensor_tensor(a[:], a[:], mask[:, 0:mw], AOP.mult)
    nc.vector.tensor_tensor(a[:], a[:], tf[:], AOP.add)
    ai = sb.tile([16, mw], I16, tag="sx_ai")
    nc.vector.tensor_copy(ai[:], a[:])
    return replicate_idx(nc, sb, ai, mw, tag + "_rep")


def load_hsel(nc, sb, hsel):
    hs = sb.tile([1, 16], I32, tag="hs")
    nc.sync.dma_start(hs[:], hsel[:].rearrange("(a b) -> a b", a=1))
    hsf = sb.tile([1, 1], F32, tag="hsf")
    nc.vector.tensor_copy(hsf[:], hs[:, 0:1])
    return hsf


# ===========================================================================
# programs
# ===========================================================================

def build_p1(nc):
    pos_pad = nc.dram_tensor("pos_pad", [3, TPAD, E], F32, kind="ExternalInput")
    val_tab = nc.dram_tensor("val_tab", [4, E], F32, kind="ExternalInput")
    dep_tab = nc.dram_tensor("dep_tab", [9, E], F32, kind="ExternalInput")
    wdr, bdr = {}, {}
    for d, nm in ((4, "4"), (5, "5"), (6, "6"), (7, "7a"), (8, "8a")):
        wdr[d] = nc.dram_tensor(f"W{nm}", [KSZ[d], E, E], F32, kind="ExternalInput")
        bdr[d] = nc.dram_tensor(f"b{nm}", [E], F32, kind="ExternalInput")

    streams = {"123": 384, "4": 512, "5": 2048, "6": 8192, "7": 16384, "8": 32768}
    vin, pin = {}, {}
    for s, m in streams.items():
        vin[s] = nc.dram_tensor(f"val_{s}", [16, m // 16], I32, kind="ExternalInput")
        pin[s] = nc.dram_tensor(f"pos_{s}", [3, 16, m // 16], I32, kind="ExternalInput")

    out123 = nc.dram_tensor("out123", [384, E], F32, kind="ExternalOutput")
    out4 = nc.dram_tensor("out4", [128, E], F32, kind="ExternalOutput")
    out5 = nc.dram_tensor("out5", [256, E], F32, kind="ExternalOutput")
    e5 = nc.dram_tensor("e5", [2048, E], BF16, kind="ExternalOutput")
    e6 = nc.dram_tensor("e6", [8192, E], BF16, kind="ExternalOutput")
    e7 = nc.dram_tensor("e7", [16384, E], BF16, kind="ExternalOutput")
    c6 = nc.dram_tensor("c6", [1024, E], BF16, kind="ExternalOutput")
    c7a = nc.dram_tensor("c7a", [2048, E], BF16, kind="ExternalOutput")
    c8 = nc.dram_tensor("c8", [4096, E], BF16, kind="ExternalOutput")

    with tile.TileContext(nc) as tc:
        with tc.tile_pool(name="sb", bufs=1) as sb, \
             tc.tile_pool(name="gat", bufs=2) as gat, \
             tc.tile_pool(name="ps", bufs=2, space="PSUM") as ps, \
             tc.tile_pool(name="dram", bufs=1, space="DRAM") as dram:

            tabs = build_tables(nc, sb, ps, dram, pos_pad, val_tab, dep_tab)
            ident = sb.tile([P, P], BF16, tag="ident")
            make_identity(nc, ident[:])

            wts = {d: load_conv_weights(nc, sb, wdr[d], KSZ[d], f"w{d}")
                   for d in range(4, 9)}
            bias = {d: load_bias(nc, sb, bdr[d], f"b{d}") for d in range(4, 9)}

            idxs = prep_gather_idx_p1(
                nc, sb, vin["123"][:], pin["123"][:], 384,
                [(1, 0, 128), (2, 128, 128), (3, 256, 128)], "i123")
            emb = gather_emb_tile(nc, gat, tabs, idxs, 0, 384, "g123")
            transpose_export(nc, gat, ps, emb, 384, out123, 0, "t123",
                             ident, out_f32=True)

            exp = {5: e5, 6: e6, 7: e7}
            cexp = {4: out4, 5: out5, 6: c6, 7: c7a, 8: c8}
            for d in range(4, 9):
                s = str(d)
                m = streams[s]
                k = KSZ[d]
                idxs = prep_gather_idx_p1(
                    nc, sb, vin[s][:], pin[s][:], m, [(d, 0, m)], f"i{s}")
                T = min(2048, m)
                for t0 in range(0, m, T):
                    emb = gather_emb_tile(nc, gat, tabs, idxs, t0, T, f"g{d}")
                    cout = conv_tile(nc, gat, ps, emb, wts[d], bias[d], k, T,
                                     f"c{d}")
                    transpose_export(nc, gat, ps, cout, T // k, cexp[d],
                                     t0 // k, f"ce{d}", ident,
                                     out_f32=(d in (4, 5)))
                    if d in exp:
                        transpose_export(nc, gat, ps, emb, T, exp[d], t0,
                                         f"ee{d}", ident, out_f32=False)
    nc.compile()


def build_p2(nc):
    src7 = nc.dram_tensor("src7", [16384 + 8192, E], BF16, kind="ExternalInput")
    src6 = nc.dram_tensor("src6", [8192 + 4096, E], BF16, kind="ExternalInput")
    val7r = nc.dram_tensor("val7r", [16, 2048], I32, kind="ExternalInput")
    val6r = nc.dram_tensor("val6r", [16, 1024], I32, kind="ExternalInput")
    hsel = nc.dram_tensor("hsel", [16], I32, kind="ExternalInput")
    w8b = nc.dram_tensor("W8b", [8, E, E], F32, kind="ExternalInput")
    b8b = nc.dram_tensor("b8b", [E], F32, kind="ExternalInput")
    w7b = nc.dram_tensor("W7b", [8, E, E], F32, kind="ExternalInput")
    b7b = nc.dram_tensor("b7b", [E], F32, kind="ExternalInput")

    c7b = nc.dram_tensor("c7b", [2048, E], BF16, kind="ExternalOutput")
    c6b = nc.dram_tensor("c6b", [1024, E], BF16, kind="ExternalOutput")

    with tile.TileContext(nc) as tc:
        with tc.tile_pool(name="sb", bufs=1) as sb, \
             tc.tile_pool(name="gat", bufs=2) as gat, \
             tc.tile_pool(name="ps", bufs=2, space="PSUM") as ps, \
             tc.tile_pool(name="ps1", bufs=1, space="PSUM") as ps1, \
             tc.tile_pool(name="dram", bufs=1, space="DRAM") as dram:
            ut16, ones16, ones1_f = make_sub_helpers(nc, sb)
            ident = sb.tile([P, P], BF16, tag="ident")
            make_identity(nc, ident[:])
            hsf = load_hsel(nc, sb, hsel)

            for (nm, src, valr, n, wd, bd, cout) in (
                    ("7", src7, val7r, 32768, w8b, b8b, c7b),
                    ("6", src6, val6r, 16384, w7b, b7b, c6b)):
                m = n // 2
                idx = sub_gather_idx(nc, sb, ps1, valr[:], n, m, m, ut16,
                                     ones16, ones1_f, hsf, f"x{nm}")
                wb = load_conv_weights(nc, sb, wd, 8, f"w{nm}")
                bias = load_bias(nc, sb, bd, f"bb{nm}")
                T = 4096
                for t0 in range(0, m, T):
                    g = gat.tile([P, 2, T], BF16, tag=f"s{nm}")
                    nc.gpsimd.dma_gather(
                        out_ap=g[:], in_ap=src[:],
                        idxs_ap=idx[:, t0 // 16:(t0 + T) // 16],
                        num_idxs=T, num_idxs_reg=T, elem_size=E,
                        transpose=True, single_packet=False,
                        queue_num=(t0 // T) % NQ)
                    co = conv_tile(nc, gat, ps, g, wb, bias, 8, T, f"cv{nm}")
                    transpose_export(nc, gat, ps, co, T // 8, cout, t0 // 8,
                                     f"ex{nm}", ident, out_f32=False)
    nc.compile()


def build_p3(nc):
    src_d6 = nc.dram_tensor("src_d6", [2048 + 2048, E], BF16, kind="ExternalInput")
    src_d7 = nc.dram_tensor("src_d7", [2048 + 2048, E], BF16, kind="ExternalInput")
    src_d8 = nc.dram_tensor("src_d8", [8192 + 4096, E], BF16, kind="ExternalInput")
    val5r = nc.dram_tensor("val5r", [16, 256], I32, kind="ExternalInput")
    val6r = nc.dram_tensor("val6r", [16, 1024], I32, kind="ExternalInput")
    hsel = nc.dram_tensor("hsel", [16], I32, kind="ExternalInput")

    o6 = nc.dram_tensor("o6", [2048, E], F32, kind="ExternalOutput")
    o7 = nc.dram_tensor("o7", [2048, E], F32, kind="ExternalOutput")
    o8 = nc.dram_tensor("o8", [8192, E], F32, kind="ExternalOutput")

    with tile.TileContext(nc) as tc:
        with tc.tile_pool(name="sb", bufs=1) as sb, \
             tc.tile_pool(name="gat", bufs=2) as gat, \
             tc.tile_pool(name="ps", bufs=2, space="PSUM") as ps, \
             tc.tile_pool(name="ps1", bufs=1, space="PSUM") as ps1, \
             tc.tile_pool(name="dram", bufs=1, space="DRAM") as dram:
            ut16, ones16, ones1_f = make_sub_helpers(nc, sb)
            hsf = load_hsel(nc, sb, hsel)

            # d6/d7 share masks over seg5 (child sources both have 2048 rows
            # after the 2048 base rows -> identical idx)
            idx5 = sub_gather_idx(nc, sb, ps1, val5r[:], 4096, 2048, 2048,
                                  ut16, ones16, ones1_f, hsf, "x5")
            idx6 = sub_gather_idx(nc, sb, ps1, val6r[:], 16384, 8192, 8192,
                                  ut16, ones16, ones1_f, hsf, "x6")

            for (nm, src, idx, m, outd) in (
                    ("6", src_d6, idx5, 2048, o6),
                    ("7", src_d7, idx5, 2048, o7),
                    ("8", src_d8, idx6, 8192, o8)):
                T = 2048
                for t0 in range(0, m, T):
                    g = gat.tile([P, T // P, E], BF16, tag=f"g{nm}")
                    nc.gpsimd.dma_gather(
                        out_ap=g[:], in_ap=src[:],
                        idxs_ap=idx[:, t0 // 16:(t0 + T) // 16],
                        num_idxs=T, num_idxs_reg=T, elem_size=E,
                        transpose=False, single_packet=False,
                        queue_num=(t0 // T) % NQ)
                    f = gat.tile([P, T // P, E], F32, tag=f"f{nm}")
                    nc.vector.tensor_copy(f[:], g[:])
                    nc.sync.dma_start(
                        outd[t0:t0 + T, :].rearrange("(c p) e -> p c e", p=P),
                        f[:])
    nc.compile()


# ===========================================================================
# host orchestration
# ===========================================================================

_PROGRAMS = {}
LAST_RESULTS = []   # BassKernelResults of the launches of the last kernel() call


def _get_program(name, builder):
    if name not in _PROGRAMS:
        nc = bacc.Bacc("TRN2", target_bir_lowering=False, debug=False)
        builder(nc)
        _PROGRAMS[name] = nc
    return _PROGRAMS[name]


def _run(nc, in_maps, **kw):
    from concourse import bass_utils
    res = bass_utils.run_bass_kernel_spmd(
        nc, in_maps, core_ids=list(range(len(in_maps))), **kw)
    LAST_RESULTS.append(res)
    return res


def _pad_tokens(v, p, target):
    """pad value slice [m] / pos slice [m,3] to target with safe values."""
    m = v.shape[0]
    if m == target:
        return v.astype(np.int32), p.astype(np.int32)
    vp = np.full((target,), 1, np.int32)
    pp = np.full((target, 3), 1, np.int32)
    vp[:m] = v
    pp[:m] = p
    return vp, pp


def _wrap16(a):
    """[m] -> [16, m/16] (token t at [t%16, t//16]); pure reshape."""
    m = a.shape[0]
    return np.ascontiguousarray(a.reshape(m // 16, 16).T).astype(np.int32)


def _wrap16_pos(p3):
    """[m, 3] -> [3, 16, m/16]."""
    return np.stack([_wrap16(p3[:, a]) for a in range(3)])


def make_p1_inputs(value, position, weights):
    """per-core (item, half) input dicts for P1."""
    pos_tab = weights["pos_tab"]
    pos_pad = np.zeros((3, TPAD, E), np.float32)
    pos_pad[:, :257, :] = np.asarray(pos_tab, np.float32)
    shared = dict(
        pos_pad=pos_pad,
        val_tab=np.asarray(weights["val_tab"], np.float32),
        dep_tab=np.asarray(weights["dep_tab"], np.float32),
        W4=np.asarray(weights["W4"], np.float32), b4=np.asarray(weights["b4"], np.float32),
        W5=np.asarray(weights["W5"], np.float32), b5=np.asarray(weights["b5"], np.float32),
        W6=np.asarray(weights["W6"], np.float32), b6=np.asarray(weights["b6"], np.float32),
        W7a=np.asarray(weights["W7a"], np.float32), b7a=np.asarray(weights["b7a"], np.float32),
        W8a=np.asarray(weights["W8a"], np.float32), b8a=np.asarray(weights["b8a"], np.float32),
    )
    in_maps = []
    for i in range(B):
        for h in range(2):
            m = dict(shared)
            vs, ps_ = [], []
            for d in (1, 2, 3):
                n = COUNTS[d - 1] // 2
                v = seg_val(value, d)[i, h * n:(h + 1) * n]
                pp_ = position[i, OFF[d - 1] + h * n:OFF[d - 1] + (h + 1) * n]
                vp, pp = _pad_tokens(v, pp_, 128)
                vs.append(vp)
                ps_.append(pp)
            m["val_123"] = _wrap16(np.concatenate(vs))
            m["pos_123"] = _wrap16_pos(np.concatenate(ps_))
            for d in range(4, 9):
                n = COUNTS[d - 1] // 2
                m[f"val_{d}"] = _wrap16(
                    seg_val(value, d)[i, h * n:(h + 1) * n])
                m[f"pos_{d}"] = _wrap16_pos(
                    position[i, OFF[d - 1] + h * n:OFF[d - 1] + (h + 1) * n])
            in_maps.append(m)
    return in_maps


def _reorder_half_first(arr, h):
    """[n] -> my half first."""
    n = arr.shape[0]
    m = n // 2
    if h == 0:
        return np.ascontiguousarray(arr)
    return np.concatenate([arr[m:], arr[:m]])


def kernel(**inputs):
    value = np.asarray(inputs["value"])
    position = np.asarray(inputs["position"])
    weights = {k: np.asarray(v) for k, v in inputs.items()
               if k not in ("value", "depth", "position")}

    LAST_RESULTS.clear()

    # ---------------- P1 ----------------
    nc1 = _get_program("p1", build_p1)
    p1_in = make_p1_inputs(value, position, weights)
    r1 = _run(nc1, p1_in).results

    # ---------------- P2 ----------------
    nc2 = _get_program("p2", build_p2)
    in2 = []
    for i in range(B):
        c8_full = np.concatenate([r1[2 * i]["c8"], r1[2 * i + 1]["c8"]])
        c7a_full = np.concatenate([r1[2 * i]["c7a"], r1[2 * i + 1]["c7a"]])
        for h in range(2):
            in2.append(dict(
                src7=np.concatenate([r1[2 * i + h]["e7"], c8_full]),
                src6=np.concatenate([r1[2 * i + h]["e6"], c7a_full]),
                val7r=_wrap16(_reorder_half_first(seg_val(value, 7)[i], h)),
                val6r=_wrap16(_reorder_half_first(seg_val(value, 6)[i], h)),
                hsel=np.full((16,), h, np.int32),
                W8b=np.asarray(weights["W8b"], np.float32),
                b8b=np.asarray(weights["b8b"], np.float32),
                W7b=np.asarray(weights["W7b"], np.float32),
                b7b=np.asarray(weights["b7b"], np.float32),
            ))
    r2 = _run(nc2, in2).results

    # ---------------- P3 ----------------
    nc3 = _get_program("p3", build_p3)
    in3 = []
    for i in range(B):
        c6_full = np.concatenate([r1[2 * i]["c6"], r1[2 * i + 1]["c6"]])
        c6b_full = np.concatenate([r2[2 * i]["c6b"], r2[2 * i + 1]["c6b"]])
        c7b_full = np.concatenate([r2[2 * i]["c7b"], r2[2 * i + 1]["c7b"]])
        for h in range(2):
            in3.append(dict(
                src_d6=np.concatenate([r1[2 * i + h]["e5"], c6_full]),
                src_d7=np.concatenate([r1[2 * i + h]["e5"], c6b_full]),
                src_d8=np.concatenate([r1[2 * i + h]["e6"], c7b_full]),
                val5r=_wrap16(_reorder_half_first(seg_val(value, 5)[i], h)),
                val6r=_wrap16(_reorder_half_first(seg_val(value, 6)[i], h)),
                hsel=np.full((16,), h, np.int32),
            ))
    r3 = _run(nc3, in3).results

    # ---------------- assemble ----------------
    out = np.zeros((B, NOUT, E), np.float32)
    for i in range(B):
        pieces = []
        for d, valid in ((1, 4), (2, 32), (3, 128)):
            j = [0, 1, 2][d - 1]
            for h in range(2):
                pieces.append(r1[2 * i + h]["out123"][j * 128:j * 128 + valid])
        for h in range(2):
            pieces.append(r1[2 * i + h]["out4"])
        for h in range(2):
            pieces.append(r1[2 * i + h]["out5"])
        for h in range(2):
            pieces.append(r3[2 * i + h]["o6"])
        for h in range(2):
            pieces.append(r3[2 * i + h]["o7"])
        for h in range(2):
            pieces.append(r3[2 * i + h]["o8"])
        out[i] = np.concatenate(pieces, axis=0)
    return out



# revision 8
# speedup vs baseline: 1.9316x; 1.9316x over previous
"""Trainium2 Bass kernel for nn_CompositeEmbeddingA (octree composite embedding).

Three SPMD launches on 8 NeuronCores (core = item*2 + half; the host only
slices / concatenates arrays and precomputes int16 gather indices between
launches — all f32 model math runs on device):

  P1: embedding gather-sum for every segment via token-major dma_gather
      (bf16) round-robined over all 4 SWDGE queues (descriptor generation
      runs on all four Q7 cpu pairs in parallel; transpose-mode gathers
      cannot be used concurrently — they corrupt each other via the shared
      xbar).  Tokens are ordered phase-major per conv tile (host-side idx
      permutation), so after PE transposes to ch-major the conv matmul
      moving operands are contiguous (3.8x faster than strided).  Embedding
      row exports (e5/e6/e7) are direct DMA copies of the gather tiles.
  P2: substituted sequences s7=sub(emb7,c8), s6=sub(emb6,c7a) assembled by
      one token-major indexed gather each (host-computed idx), then convs
      W8b/W7b via the same transpose+matmul path.
  P3: final d6/d7/d8 rows assembled by token-major indexed gathers, upcast
      to f32.

Returns the full [4, 25672, 256] f32 output.
"""

import numpy as np
import ml_dtypes

import concourse.bass as bass
import concourse.tile as tile
from concourse import bacc, mybir
from concourse.masks import make_identity

BF16 = mybir.dt.bfloat16
F32 = mybir.dt.float32
I16 = mybir.dt.int16
AOP = mybir.AluOpType
ACT_IDENT = mybir.ActivationFunctionType.Identity
NPBF16 = ml_dtypes.bfloat16

P = 128
E = 256
B = 4
COUNTS = [8, 64, 256, 1024, 4096, 16384, 32768, 65536]
OFF = [0, 8, 72, 328, 1352, 5448, 21832, 54600, 120136]
NOUT = 25672
KSZ = {4: 4, 5: 8, 6: 8, 7: 8, 8: 8}
TPAD = 384
T_TILE = 4096
STREAM_M = {4: 512, 5: 2048, 6: 8192, 7: 16384, 8: 32768}  # tokens per core


def seg_val(a, d):
    return a[:, OFF[d - 1]:OFF[d]]


# ===========================================================================
# host-side index helpers
# ===========================================================================

def _perm_for(m, k, T):
    """gather-output position p -> token index (phase-major within tiles)."""
    out = np.empty(m, np.int64)
    for t0 in range(0, m, T):
        Tc = min(T, m - t0)
        G = Tc // k
        ar = np.arange(Tc)
        out[t0:t0 + Tc] = t0 + (ar % G) * k + ar // G
    return out


def _invperm(p):
    inv = np.empty_like(p)
    inv[p] = np.arange(len(p))
    return inv


PERMS = {d: _perm_for(STREAM_M[d], KSZ[d], T_TILE) for d in range(4, 9)}
INVPERMS = {d: _invperm(PERMS[d]) for d in range(4, 9)}
PERM_S7 = _perm_for(16384, 8, T_TILE)   # P2 conv over s7
PERM_S6 = _perm_for(8192, 8, T_TILE)    # P2 conv over s6


def _wrap16rep(a):
    """[m] int -> [128, m/16] int16 (token j at [j%16, j//16], replicated)."""
    m = a.shape[0]
    w = np.ascontiguousarray(a.reshape(m // 16, 16).T)
    return np.tile(w, (8, 1)).astype(np.int16)


def _sub_idx(value, d, i, h, child_off, apply_perm):
    """Host idx for substitution gather from [base_perm_rows | child_full]."""
    m = COUNTS[d - 1] // 2
    vfull = np.asarray(seg_val(value, d)[i]).astype(np.int64)
    mask = vfull == 2
    crank = np.cumsum(mask) - 1
    tloc = np.arange(m)
    tglob = h * m + tloc
    idx = np.where(mask[tglob], child_off + crank[tglob], INVPERMS[d][tloc])
    if apply_perm is not None:
        idx = idx[apply_perm]
    return _wrap16rep(idx)


# ===========================================================================
# device-side helpers
# ===========================================================================

class QS:
    """Strict cyclic SWDGE queue assignment; q0 last (it blocks the engine)."""
    def __init__(self):
        self.i = 0

    def pick(self):
        q = (1, 2, 3, 0)[self.i % 4]
        self.i += 1
        return q


def load_bias(nc, sb, b_dram, tag):
    bt = sb.tile([P, 2], F32, tag=tag)
    nc.sync.dma_start(bt[:], b_dram[:].rearrange("(m p) -> p m", p=P))
    return bt


def tok_to_ch(nc, embp, ps, g, T, ident, tag):
    """g [128, T/128, 256] bf16 token-major -> [128, 2, T] bf16 ch-major.
    Tokens stay in tile order (phase-major), so conv moving slices are
    contiguous."""
    nblk = T // P
    embc = embp.tile([P, 2, T], BF16, tag=tag)
    ci = 0
    for j in range(2):
        for c0 in range(0, nblk, 4):
            nb = min(4, nblk - c0)
            tin = ps.tile([P, 4, P], BF16, tag="tin")
            for b in range(nb):
                nc.tensor.transpose(
                    tin[:, b, :], g[:, c0 + b, j * P:(j + 1) * P], ident[:])
            eng = nc.scalar if (ci % 2 == 0) else nc.vector
            if ci % 2 == 0:
                nc.scalar.copy(embc[:, j, c0 * P:(c0 + nb) * P],
                               tin[:, :nb, :])
            else:
                nc.vector.tensor_copy(embc[:, j, c0 * P:(c0 + nb) * P],
                                      tin[:, :nb, :])
            ci += 1
    return embc


def conv_tile(nc, pool, ps, emb, wb, bias, k, T):
    """emb [128, 2, T] bf16 ch-major phase-major -> out bf16 ch-major
    [128, 2, T/k] (natural group order)."""
    G = T // k
    out = pool.tile([P, 2, G], BF16, tag="co")
    for mo in range(2):
        pt = ps.tile([P, G], F32, tag="cps")
        n_mm = 2 * k
        i = 0
        for j in range(2):
            for ph in range(k):
                nc.tensor.matmul(
                    pt[:], wb[:, ph, j, mo, :], emb[:, j, ph * G:(ph + 1) * G],
                    start=(i == 0), stop=(i == n_mm - 1))
                i += 1
        nc.scalar.activation(
            out[:, mo, :], pt[:], ACT_IDENT, bias=bias[:, mo:mo + 1], scale=1.0)
    return out


def transpose_export(nc, pool, ps, src, n_tok, dram_out, row_off, ident,
                     out_f32=False):
    """src [128, 2, n_tok] bf16 ch-major -> token-major rows into
    dram_out[row_off:row_off+n_tok].  n_tok % 128 == 0."""
    nblk = n_tok // P
    for c0 in range(0, nblk, 4):
        nb = min(4, nblk - c0)
        pt = ps.tile([P, 4, E], BF16, tag="tps")
        for bi in range(nb):
            for j in range(2):
                nc.tensor.transpose(
                    pt[:, bi, j * P:(j + 1) * P],
                    src[:, j, (c0 + bi) * P:(c0 + bi + 1) * P],
                    ident[:])
        st = pool.tile([P, 4, E], F32 if out_f32 else BF16,
                       tag="st_f" if out_f32 else "st_b")
        nc.scalar.copy(st[:, :nb, :], pt[:, :nb, :])
        nc.sync.dma_start(
            dram_out[row_off + c0 * P: row_off + (c0 + nb) * P, :].rearrange(
                "(c p) e -> p c e", p=P),
            st[:, :nb, :])


def build_tables(nc, sb, psb, dram, pos_pad, val_tab, dep_tab, depth_order):
    """Build bf16 gather tables in device DRAM:
       t1, t2 [384, 256]: pos_tab[1], pos_tab[2]
       t0[d] [1152, 256]: row (v-1)*384+p = val[v]+dep[d]+pos0[p], d in 4..8
       t0_123 [3456, 256]: depths 1..3 concatenated.
    Ordered so deeper (bigger) streams become ready first."""
    pos_sb = []
    for a in range(3):
        t = sb.tile([P, 3, E], F32, tag=f"pos_stage{a}")
        nc.sync.dma_start(t[:], pos_pad[a].rearrange("(j p) e -> p j e", p=P))
        pos_sb.append(t)

    t1 = dram.tile([TPAD, E], BF16)
    t2 = dram.tile([TPAD, E], BF16)
    for a, tdst in ((1, t1), (2, t2)):
        tb = sb.tile([P, 3, E], BF16, tag=f"pos_bf{a}")
        nc.vector.tensor_copy(tb[:], pos_sb[a][:])
        nc.sync.dma_start(tdst[:].rearrange("(j p) e -> p j e", p=P), tb[:])

    vt = sb.tile([1, 4 * E], F32, tag="vt")
    dt_ = sb.tile([1, 9 * E], F32, tag="dt")
    nc.sync.dma_start(vt[:], val_tab[:].rearrange("v e -> (v e)"))
    nc.sync.dma_start(dt_[:], dep_tab[:].rearrange("v e -> (v e)"))

    ones_f = sb.tile([1, P], F32, tag="ones_f")
    nc.vector.memset(ones_f[:], 1.0)

    t0 = {d: dram.tile([3 * TPAD, E], BF16, name=f"t0_{d}", tag=f"t0_{d}")
          for d in range(4, 9)}
    t0_123 = dram.tile([3 * 3 * TPAD, E], BF16, name="t0_123", tag="t0_123")

    def build_block(d, dst, row0):
        stage = sb.tile([P, 9, E], BF16, tag="t0_stage")
        for v in range(1, 4):
            vd = sb.tile([1, E], F32, tag="vd_tmp")
            nc.vector.tensor_tensor(
                vd[:], vt[:, v * E:(v + 1) * E], dt_[:, d * E:(d + 1) * E],
                AOP.add)
            bc = psb.tile([P, E], F32, tag="bc_ps")
            nc.tensor.matmul(bc[:], ones_f[:], vd[:], start=True, stop=True)
            for j in range(3):
                nc.vector.tensor_tensor(
                    stage[:, 3 * (v - 1) + j, :], pos_sb[0][:, j, :], bc[:],
                    AOP.add)
        nc.sync.dma_start(
            dst[row0:row0 + 3 * TPAD, :].rearrange("(j p) e -> p j e", p=P),
            stage[:])

    for d in depth_order:
        build_block(d, t0[d], 0)
    for d in (1, 2, 3):
        build_block(d, t0_123, (d - 1) * 3 * TPAD)
    return t0, t0_123, t1, t2


def gather_tokmajor(nc, pool, tab, idx_ap, T, q, tag):
    g = pool.tile([P, T // P, E], BF16, tag=tag)
    nc.gpsimd.dma_gather(
        out_ap=g[:], in_ap=tab[:], idxs_ap=idx_ap,
        num_idxs=T, num_idxs_reg=T, elem_size=E,
        transpose=False, single_packet=False, queue_num=q)
    return g


# ===========================================================================
# programs
# ===========================================================================

def build_p1(nc):
    pos_pad = nc.dram_tensor("pos_pad", [3, TPAD, E], F32, kind="ExternalInput")
    val_tab = nc.dram_tensor("val_tab", [4, E], F32, kind="ExternalInput")
    dep_tab = nc.dram_tensor("dep_tab", [9, E], F32, kind="ExternalInput")
    wdr, bdr = {}, {}
    for d, nm in ((4, "4"), (5, "5"), (6, "6"), (7, "7a"), (8, "8a")):
        wdr[d] = nc.dram_tensor(f"W{nm}", [P, KSZ[d], 2, 2, P], BF16,
                                kind="ExternalInput")
        bdr[d] = nc.dram_tensor(f"b{nm}", [E], F32, kind="ExternalInput")

    idx_in = {}
    for s, m in (("123", 384),) + tuple((str(d), STREAM_M[d]) for d in range(4, 9)):
        idx_in[s] = nc.dram_tensor(f"idx_{s}", [3, P, m // 16], I16,
                                   kind="ExternalInput")

    out123 = nc.dram_tensor("out123", [384, E], F32, kind="ExternalOutput")
    out4 = nc.dram_tensor("out4", [128, E], F32, kind="ExternalOutput")
    out5 = nc.dram_tensor("out5", [256, E], F32, kind="ExternalOutput")
    e5 = nc.dram_tensor("e5", [2048, E], BF16, kind="ExternalOutput")
    e6 = nc.dram_tensor("e6", [8192, E], BF16, kind="ExternalOutput")
    e7 = nc.dram_tensor("e7", [16384, E], BF16, kind="ExternalOutput")
    c6 = nc.dram_tensor("c6", [1024, E], BF16, kind="ExternalOutput")
    c7a = nc.dram_tensor("c7a", [2048, E], BF16, kind="ExternalOutput")
    c8 = nc.dram_tensor("c8", [4096, E], BF16, kind="ExternalOutput")

    qs = QS()
    with tile.TileContext(nc) as tc:
        with tc.tile_pool(name="sb", bufs=1) as sb, \
             tc.tile_pool(name="gat", bufs=2) as gat, \
             tc.tile_pool(name="embp", bufs=2) as embp, \
             tc.tile_pool(name="wp", bufs=2) as wp, \
             tc.tile_pool(name="ixp", bufs=1) as ixp, \
             tc.tile_pool(name="ps", bufs=2, space="PSUM") as ps, \
             tc.tile_pool(name="ps1", bufs=1, space="PSUM") as ps1, \
             tc.tile_pool(name="dram", bufs=1, space="DRAM") as dram:

            t0, t0_123, t1, t2 = build_tables(
                nc, sb, ps1, dram, pos_pad, val_tab, dep_tab, (8, 7, 6, 5, 4))
            ident = sb.tile([P, P], BF16, tag="ident")
            make_identity(nc, ident[:])

            exp = {5: e5, 6: e6, 7: e7}
            cexp = {4: out4, 5: out5, 6: c6, 7: c7a, 8: c8}
            bias = {d: load_bias(nc, sb, bdr[d], f"b{d}") for d in range(4, 9)}
            for d in (8, 7, 6, 5, 4):
                s = str(d)
                m = STREAM_M[d]
                k = KSZ[d]
                T = min(T_TILE, m)
                w16 = m // 16
                wb = wp.tile([P, KSZ[d], 2, 2, P], BF16, tag="wcur")
                nc.sync.dma_start(wb[:], wdr[d][:])
                idx = ixp.tile([P, 3, w16], I16, tag="ixcur")
                nc.sync.dma_start(idx[:], idx_in[s][:].rearrange("a p w -> p a w"))
                for t0tok in range(0, m, T):
                    c0, c1 = t0tok // 16, (t0tok + T) // 16
                    g = []
                    for a, tab in enumerate((t0[d], t1, t2)):
                        g.append(gather_tokmajor(
                            nc, gat, tab, idx[:, a, c0:c1], T, qs.pick(),
                            f"gg{a}"))
                    nc.vector.tensor_tensor(g[0][:], g[0][:], g[1][:], AOP.add)
                    nc.vector.tensor_tensor(g[0][:], g[0][:], g[2][:], AOP.add)
                    if d in exp:
                        nc.sync.dma_start(
                            exp[d][t0tok:t0tok + T, :].rearrange(
                                "(c p) e -> p c e", p=P),
                            g[0][:])
                    embc = tok_to_ch(nc, embp, ps, g[0], T, ident, "embc")
                    co = conv_tile(nc, gat, ps, embc, wb, bias[d], k, T)
                    transpose_export(nc, gat, ps, co, T // k, cexp[d],
                                     t0tok // k, ident, out_f32=(d in (4, 5)))

            # streams 1-3 (padded to 384 tokens, identity embedding)
            idx = ixp.tile([P, 3, 24], I16, tag="ixcur")
            nc.sync.dma_start(idx[:], idx_in["123"][:].rearrange("a p w -> p a w"))
            g = []
            for a, tab in enumerate((t0_123, t1, t2)):
                g.append(gather_tokmajor(
                    nc, gat, tab, idx[:, a, :], 384, qs.pick(), f"gg{a}"))
            nc.vector.tensor_tensor(g[0][:], g[0][:], g[1][:], AOP.add)
            nc.vector.tensor_tensor(g[0][:], g[0][:], g[2][:], AOP.add)
            gf = gat.tile([P, 3, E], F32, tag="st_f")
            nc.vector.tensor_copy(gf[:], g[0][:])
            nc.sync.dma_start(
                out123[:].rearrange("(c p) e -> p c e", p=P), gf[:])
    nc.compile()


def build_p2(nc):
    src7 = nc.dram_tensor("src7", [16384 + 8192, E], BF16, kind="ExternalInput")
    src6 = nc.dram_tensor("src6", [8192 + 4096, E], BF16, kind="ExternalInput")
    idx7 = nc.dram_tensor("idx7", [P, 1024], I16, kind="ExternalInput")
    idx6 = nc.dram_tensor("idx6", [P, 512], I16, kind="ExternalInput")
    w8b = nc.dram_tensor("W8b", [P, 8, 2, 2, P], BF16, kind="ExternalInput")
    b8b = nc.dram_tensor("b8b", [E], F32, kind="ExternalInput")
    w7b = nc.dram_tensor("W7b", [P, 8, 2, 2, P], BF16, kind="ExternalInput")
    b7b = nc.dram_tensor("b7b", [E], F32, kind="ExternalInput")

    c7b = nc.dram_tensor("c7b", [2048, E], BF16, kind="ExternalOutput")
    c6b = nc.dram_tensor("c6b", [1024, E], BF16, kind="ExternalOutput")

    qs = QS()
    with tile.TileContext(nc) as tc:
        with tc.tile_pool(name="sb", bufs=1) as sb, \
             tc.tile_pool(name="gat", bufs=2) as gat, \
             tc.tile_pool(name="embp", bufs=2) as embp, \
             tc.tile_pool(name="ps", bufs=2, space="PSUM") as ps:
            ident = sb.tile([P, P], BF16, tag="ident")
            make_identity(nc, ident[:])

            for (nm, src, idxd, m, wd, bd, cout) in (
                    ("7", src7, idx7, 16384, w8b, b8b, c7b),
                    ("6", src6, idx6, 8192, w7b, b7b, c6b)):
                wb = sb.tile([P, 8, 2, 2, P], BF16, tag=f"w{nm}")
                nc.sync.dma_start(wb[:], wd[:])
                bias = load_bias(nc, sb, bd, f"bb{nm}")
                idx = sb.tile([P, m // 16], I16, tag=f"ix{nm}")
                nc.sync.dma_start(idx[:], idxd[:])
                T = T_TILE
                for t0 in range(0, m, T):
                    g = gather_tokmajor(
                        nc, gat, src, idx[:, t0 // 16:(t0 + T) // 16], T,
                        qs.pick(), "sg")
                    embc = tok_to_ch(nc, embp, ps, g, T, ident, "embc")
                    co = conv_tile(nc, gat, ps, embc, wb, bias, 8, T)
                    transpose_export(nc, gat, ps, co, T // 8, cout, t0 // 8,
                                     ident, out_f32=False)
    nc.compile()


def build_p3(nc):
    src_d6 = nc.dram_tensor("src_d6", [2048 + 2048, E], BF16, kind="ExternalInput")
    src_d7 = nc.dram_tensor("src_d7", [2048 + 2048, E], BF16, kind="ExternalInput")
    src_d8 = nc.dram_tensor("src_d8", [8192 + 4096, E], BF16, kind="ExternalInput")
    idx56 = nc.dram_tensor("idx56", [P, 128], I16, kind="ExternalInput")
    idx8 = nc.dram_tensor("idx8", [P, 512], I16, kind="ExternalInput")

    o6 = nc.dram_tensor("o6", [2048, E], F32, kind="ExternalOutput")
    o7 = nc.dram_tensor("o7", [2048, E], F32, kind="ExternalOutput")
    o8 = nc.dram_tensor("o8", [8192, E], F32, kind="ExternalOutput")

    qs = QS()
    with tile.TileContext(nc) as tc:
        with tc.tile_pool(name="sb", bufs=1) as sb, \
             tc.tile_pool(name="gat", bufs=2) as gat:
            i56 = sb.tile([P, 128], I16, tag="i56")
            nc.sync.dma_start(i56[:], idx56[:])
            i8 = sb.tile([P, 512], I16, tag="i8")
            nc.sync.dma_start(i8[:], idx8[:])

            for (nm, src, idx, m, outd) in (
                    ("8", src_d8, i8, 8192, o8),
                    ("6", src_d6, i56, 2048, o6),
                    ("7", src_d7, i56, 2048, o7)):
                T = 2048
                for t0 in range(0, m, T):
                    g = gather_tokmajor(
                        nc, gat, src, idx[:, t0 // 16:(t0 + T) // 16], T,
                        qs.pick(), f"g{nm}")
                    f = gat.tile([P, T // P, E], F32, tag=f"f{nm}")
                    nc.vector.tensor_copy(f[:], g[:])
                    nc.sync.dma_start(
                        outd[t0:t0 + T, :].rearrange("(c p) e -> p c e", p=P),
                        f[:])
    nc.compile()


# ===========================================================================
# host orchestration
# ===========================================================================

_PROGRAMS = {}
LAST_RESULTS = []   # BassKernelResults of the launches of the last kernel() call


def _get_program(name, builder):
    if name not in _PROGRAMS:
        nc = bacc.Bacc("TRN2", target_bir_lowering=False, debug=False,
                       num_swdge_queues=4)
        builder(nc)
        _PROGRAMS[name] = nc
    return _PROGRAMS[name]


def _run(nc, in_maps, **kw):
    from concourse import bass_utils
    res = bass_utils.run_bass_kernel_spmd(
        nc, in_maps, core_ids=list(range(len(in_maps))), **kw)
    LAST_RESULTS.append(res)
    return res


def _wconv(W):
    """W [k,256,256] f32 -> [128, k, 2, 2, 128] bf16 (device wb layout)."""
    k = W.shape[0]
    w = np.asarray(W, np.float32).reshape(k, 2, P, 2, P)   # k, j, p, m, c
    w = np.transpose(w, (2, 0, 1, 3, 4))                   # p, k, j, m, c
    return np.ascontiguousarray(w).astype(NPBF16)


def make_p1_inputs(value, position, weights):
    pos_tab = weights["pos_tab"]
    pos_pad = np.zeros((3, TPAD, E), np.float32)
    pos_pad[:, :257, :] = np.asarray(pos_tab, np.float32)
    shared = dict(
        pos_pad=pos_pad,
        val_tab=np.asarray(weights["val_tab"], np.float32),
        dep_tab=np.asarray(weights["dep_tab"], np.float32),
        W4=_wconv(weights["W4"]), b4=np.asarray(weights["b4"], np.float32),
        W5=_wconv(weights["W5"]), b5=np.asarray(weights["b5"], np.float32),
        W6=_wconv(weights["W6"]), b6=np.asarray(weights["b6"], np.float32),
        W7a=_wconv(weights["W7a"]), b7a=np.asarray(weights["b7a"], np.float32),
        W8a=_wconv(weights["W8a"]), b8a=np.asarray(weights["b8a"], np.float32),
    )
    value = np.asarray(value)
    position = np.asarray(position)
    in_maps = []
    for i in range(B):
        for h in range(2):
            m = dict(shared)
            # streams 4..8: permuted idx triples
            for d in range(4, 9):
                n = STREAM_M[d]
                v = np.asarray(seg_val(value, d)[i, h * n:(h + 1) * n]).astype(np.int64)
                p = np.asarray(position[i, OFF[d - 1] + h * n:
                                        OFF[d - 1] + (h + 1) * n]).astype(np.int64)
                pm = PERMS[d]
                i0 = ((v - 1) * TPAD + p[:, 0])[pm]
                i1 = p[:, 1][pm]
                i2 = p[:, 2][pm]
                m[f"idx_{d}"] = np.stack(
                    [_wrap16rep(i0), _wrap16rep(i1), _wrap16rep(i2)])
            # stream 123 (pad each depth to 128)
            i0l, i1l, i2l = [], [], []
            for d in (1, 2, 3):
                n = COUNTS[d - 1] // 2
                v = np.asarray(seg_val(value, d)[i, h * n:(h + 1) * n]).astype(np.int64)
                p = np.asarray(position[i, OFF[d - 1] + h * n:
                                        OFF[d - 1] + (h + 1) * n]).astype(np.int64)
                a0 = np.full(128, 1, np.int64)
                a1 = np.full(128, 1, np.int64)
                a2 = np.full(128, 1, np.int64)
                a0[:n] = (d - 1) * 3 * TPAD + (v - 1) * TPAD + p[:, 0]
                a1[:n] = p[:, 1]
                a2[:n] = p[:, 2]
                i0l.append(a0)
                i1l.append(a1)
                i2l.append(a2)
            m["idx_123"] = np.stack([_wrap16rep(np.concatenate(i0l)),
                                     _wrap16rep(np.concatenate(i1l)),
                                     _wrap16rep(np.concatenate(i2l))])
            in_maps.append(m)
    return in_maps


def kernel(**inputs):
    value = np.asarray(inputs["value"])
    position = np.asarray(inputs["position"])
    weights = {k: np.asarray(v) for k, v in inputs.items()
               if k not in ("value", "depth", "position")}

    LAST_RESULTS.clear()

    # ---------------- P1 ----------------
    nc1 = _get_program("p1", build_p1)
    r1 = _run(nc1, make_p1_inputs(value, position, weights)).results

    # ---------------- P2 ----------------
    nc2 = _get_program("p2", build_p2)
    w8b, b8b = _wconv(weights["W8b"]), np.asarray(weights["b8b"], np.float32)
    w7b, b7b = _wconv(weights["W7b"]), np.asarray(weights["b7b"], np.float32)
    in2 = []
    for i in range(B):
        c8_full = np.concatenate([r1[2 * i]["c8"], r1[2 * i + 1]["c8"]])
        c7a_full = np.concatenate([r1[2 * i]["c7a"], r1[2 * i + 1]["c7a"]])
        for h in range(2):
            in2.append(dict(
                src7=np.concatenate([r1[2 * i + h]["e7"], c8_full]),
                src6=np.concatenate([r1[2 * i + h]["e6"], c7a_full]),
                idx7=_sub_idx(value, 7, i, h, 16384, PERM_S7),
                idx6=_sub_idx(value, 6, i, h, 8192, PERM_S6),
                W8b=w8b, b8b=b8b, W7b=w7b, b7b=b7b,
            ))
    r2 = _run(nc2, in2).results

    # ---------------- P3 ----------------
    nc3 = _get_program("p3", build_p3)
    in3 = []
    for i in range(B):
        c6_full = np.concatenate([r1[2 * i]["c6"], r1[2 * i + 1]["c6"]])
        c6b_full = np.concatenate([r2[2 * i]["c6b"], r2[2 * i + 1]["c6b"]])
        c7b_full = np.concatenate([r2[2 * i]["c7b"], r2[2 * i + 1]["c7b"]])
        for h in range(2):
            in3.append(dict(
                src_d6=np.concatenate([r1[2 * i + h]["e5"], c6_full]),
                src_d7=np.concatenate([r1[2 * i + h]["e5"], c6b_full]),
                src_d8=np.concatenate([r1[2 * i + h]["e6"], c7b_full]),
                idx56=_sub_idx(value, 5, i, h, 2048, None),
                idx8=_sub_idx(value, 6, i, h, 8192, None),
            ))
    r3 = _run(nc3, in3).results

    # ---------------- assemble ----------------
    out = np.zeros((B, NOUT, E), np.float32)
    for i in range(B):
        pieces = []
        for d, valid in ((1, 4), (2, 32), (3, 128)):
            j = d - 1
            for h in range(2):
                pieces.append(r1[2 * i + h]["out123"][j * 128:j * 128 + valid])
        for h in range(2):
            pieces.append(r1[2 * i + h]["out4"])
        for h in range(2):
            pieces.append(r1[2 * i + h]["out5"])
        for h in range(2):
            pieces.append(r3[2 * i + h]["o6"])
        for h in range(2):
            pieces.append(r3[2 * i + h]["o7"])
        for h in range(2):
            pieces.append(r3[2 * i + h]["o8"])
        out[i] = np.concatenate(pieces, axis=0)
    return out
